# revision 14
# baseline (speedup 1.0000x reference)
"""AdditiveAttention (FastFormer-style) Trainium2 kernel.

Strategy
--------
Data-parallel over batch: B=8 batch elements -> 8 NeuronCores, one element
per core, no collectives. Per core the computation is four 4096x1024x1024
GEMMs (q/k/v/out projections) plus two softmax-over-token poolings and
cheap elementwise stages.

Device layout is feature-major ("transposed"): every tensor is [d, t] with
feature channels on SBUF partitions and tokens on the free axis, so
- projections contract over the partition axis (natural PE matmuls),
- softmax over tokens is a free-axis reduction (natural DVE/ACT ops),
- the pooled vectors q_global/k_global become per-partition scalars, so the
  broadcast multiplies are per-partition `activation(scale=...)` ops.

Feature channels are permuted (host-side) so each 128-partition block
contains 8 channels of each of the 16 heads; one [128, T] replication of the
16 softmax weight rows then serves every block.

Algebra: the reference's p = k * q_global tensor is never materialized:
    beta     = (wk ⊙ q_global) @ k_tilde        (constant shifts cancel in
                                                 softmax over tokens)
    k_global = q_global ⊙ pool(k_tilde, betas)
All zero-initialized biases of the module are still handled exactly (they
fold into epilogue bias vectors / tiny [128, 8] fixups).

Compute dtype is bf16 (fp32 PSUM accumulation). The output of this module is
q + (attention correction), and the correction is ~4e-4 of the output norm,
so output accuracy is set by the q-projection path; bf16 gives ~1e-3
relative error overall.
"""

import sys

if "/opt/trn_rl_repo" not in sys.path:
    sys.path.insert(0, "/opt/trn_rl_repo")

import numpy as np
import ml_dtypes

import bass_rust
import concourse.bass as bass
import concourse.tile as tile
from concourse import mybir
from concourse.bass_utils import run_bass_kernel_spmd

BF16 = mybir.dt.bfloat16
F32 = mybir.dt.float32
NPBF16 = ml_dtypes.bfloat16

B, S, D = 8, 4096, 1024
H, DH = 16, 64
NB = 8          # feature blocks of 128
NCH = 8         # token chunks
CH = S // NCH   # 512
N_CORES = 8


def _patched_drain_and_barrier(self, tick_clock, wait_clock):
    # The pinned walrus build only accepts ONE sync wait on a Drain
    # instruction; split the kernel-tail drain's waits across a chain.
    drain_inst = self.nc.sync.drain()
    wait_clock.add_sem_waits(
        drain_inst.ins, tile.ScopedClock({None: tick_clock.global_clock})
    )
    si = drain_inst.ins.sync_info
    waits = list(si.on_wait)
    if len(waits) > 1:
        si.on_wait = waits[:1]
        for w in waits[1:]:
            extra = self.nc.sync.drain()
            extra.ins.sync_info = bass_rust.SyncInfo(on_wait=[w], on_update=[])
    self.nc.all_engine_barrier()
    popped = self.nc._tile_sem_poison_stack.pop()
    assert popped is self._sem_poison
    self.nc.clear_and_free_semaphores(list(self.sems.allocated().values()))
    self.nc.all_engine_barrier()


tile.TileContext._drain_and_barrier = _patched_drain_and_barrier

GATE_NAME = "waitgate"


def legalize_waits(nc):
    """The pinned walrus accepts at most ONE sync wait per instruction,
    while Tile freely emits several. Three-step legalization:

    1) transitive elision: drop waits already implied through the vector-
       clock closure of the instruction's proc + its other waits (Tile's
       own elision is per-proc only, not transitive);
    2) engine instructions: move surplus waits onto preceding NoOps on the
       same engine (in-order sequencers make this exactly equivalent);
    3) DMAs (queue-descriptor waits, not sequencer-evaluated): funnel all
       waits through a chain of Pool-engine NoOps that increments a
       dedicated gate semaphore; the DMA then waits on the gate count.
    """
    f = nc.m.functions[0]

    # pick a gate sem id above everything Tile allocated, and extend the
    # kernel-tail sem reset range to cover it
    used_ids = set()
    for blk in f.blocks:
        for inst in blk.instructions:
            si = inst.sync_info
            if si:
                for x in list(si.on_wait) + list(si.on_update):
                    used_ids.add(x.id)
            try:
                if inst.reset_range_stop is not None:
                    used_ids.add(inst.reset_range_stop - 1)
            except AttributeError:
                pass
    gate_id = max(used_ids) + 1
    n_ext = 0
    for blk in f.blocks:
        for inst in blk.instructions:
            try:
                rs = inst.reset_range_stop
            except AttributeError:
                continue
            if rs is not None and rs > 155 and rs <= gate_id:
                inst.reset_range_stop = gate_id + 1
                n_ext += 1
    assert n_ext >= 1, "no sem reset range found to extend"

    # ---- pass 1: transitive elision over the scheduled stream ----
    sem_hist = {}
    sem_cum = {}
    sem_dirty = set()
    proc_clock = {}

    def proc_of(inst):
        if inst.opcode == "DMACopy":
            si = inst.sync_info
            ups = list(si.on_update) if si else []
            if ups:
                return "Q:" + ups[0].ant_name
        return "E:" + str(inst.engine)

    def merge(a, b):
        for k, v in b.items():
            if a.get(k, -1) < v:
                a[k] = v

    def implied(w):
        if w.ant_name in sem_dirty:
            return None
        for cum, clk in sem_hist.get(w.ant_name, []):
            if cum >= w.wait_value:
                return clk
        return None

    for blk in f.blocks:
        for inst in blk.instructions:
            si = inst.sync_info
            waits = list(si.on_wait) if si else []
            P = proc_of(inst)
            pc = proc_clock.setdefault(P, {})
            ge = [w for w in waits
                  if w.wait_mode == "sem-ge-imm" and w.wait_reg is None]
            other = [w for w in waits
                     if not (w.wait_mode == "sem-ge-imm" and w.wait_reg is None)]
            needed = list(ge)
            changed = True
            while changed and len(needed) + len(other) > 1:
                changed = False
                for w in list(needed):
                    base = dict(pc)
                    for w2 in needed:
                        if w2 is w:
                            continue
                        ic = implied(w2)
                        if ic:
                            merge(base, ic)
                    if base.get(w.ant_name, -1) >= w.wait_value:
                        needed.remove(w)
                        changed = True
                        break
            if si is not None and len(needed) + len(other) != len(waits):
                si.on_wait = other + needed
            for w in ge:
                ic = implied(w)
                if ic:
                    merge(pc, ic)
                if pc.get(w.ant_name, -1) < w.wait_value:
                    pc[w.ant_name] = w.wait_value
            ups = list(si.on_update) if si else []
            comp = dict(pc)
            for u in ups:
                if u.update_mode == "sem-inc" and u.ant_name not in sem_dirty:
                    sem_cum[u.ant_name] = sem_cum.get(u.ant_name, 0) + u.update_value
                    comp[u.ant_name] = sem_cum[u.ant_name]
                else:
                    sem_dirty.add(u.ant_name)
            for u in ups:
                if u.update_mode == "sem-inc" and u.ant_name not in sem_dirty:
                    sem_hist.setdefault(u.ant_name, []).append(
                        (sem_cum[u.ant_name], comp)
                    )
            proc_clock[P] = pc

    # ---- pass 2/3: split survivors ----
    gate_n = 0
    nop_n = 0
    n_split = 0
    for blk in f.blocks:
        out = []
        changed = False
        for inst in blk.instructions:
            si = inst.sync_info
            waits = list(si.on_wait) if si else []
            if len(waits) <= 1:
                out.append(inst)
                continue
            changed = True
            n_split += 1
            if inst.opcode == "DMACopy":
                for w in waits:
                    nop_n += 1
                    nop = bass_rust.InstNoOp(name=f"gz{nop_n}")
                    nop.engine = mybir.EngineType.Pool
                    upd = []
                    if w is waits[-1]:
                        gate_n += 1
                        upd = [bass_rust.SyncUpdate(
                            sync_type="semaphore", id=gate_id,
                            ant_name=GATE_NAME, update_mode="sem-inc",
                            update_value=1)]
                    nop.sync_info = bass_rust.SyncInfo(on_wait=[w], on_update=upd)
                    out.append(nop)
                si.on_wait = [bass_rust.SyncWait(
                    sync_type="semaphore", id=gate_id, ant_name=GATE_NAME,
                    wait_mode="sem-ge-imm", wait_value=gate_n, wait_reg=None)]
                out.append(inst)
            else:
                for w in waits[:-1]:
                    nop_n += 1
                    nop = bass_rust.InstNoOp(name=f"wz{nop_n}")
                    nop.engine = inst.engine
                    nop.sync_info = bass_rust.SyncInfo(on_wait=[w], on_update=[])
                    out.append(nop)
                si.on_wait = [waits[-1]]
                out.append(inst)
        if changed:
            blk.instructions = out
    print(f"legalize_waits: {n_split} multi-wait instructions split "
          f"({gate_n} DMA gates, {nop_n} nops)")


def _perm_idx():
    # position (block i, partition p) holds original channel
    # (p % 16) * 64 + i * 8 + p // 16  ->  head(position) == p % 16 for all i
    j = np.arange(D)
    i, p = j // 128, j % 128
    idx = (p % 16) * 64 + i * 8 + p // 16
    assert np.array_equal(np.sort(idx), np.arange(D))
    return idx


P_IDX = _perm_idx()


def build_kernel():
    nc = bass.Bass()

    xq_e = nc.declare_dram_parameter("xq", [D, S], BF16, isOutput=False)
    xkv_e = nc.declare_dram_parameter("xkv", [D, S], BF16, isOutput=False)
    qw_e = nc.declare_dram_parameter("qw", [D, D], BF16, isOutput=False)
    kw_e = nc.declare_dram_parameter("kw", [D, D], BF16, isOutput=False)
    vw_e = nc.declare_dram_parameter("vw", [D, D], BF16, isOutput=False)
    ow_e = nc.declare_dram_parameter("ow", [D, D], BF16, isOutput=False)
    wqx_e = nc.declare_dram_parameter("wqx", [D, H], BF16, isOutput=False)
    wks_e = nc.declare_dram_parameter("wks", [D, H], BF16, isOutput=False)
    qob_e = nc.declare_dram_parameter("qob", [128, NB], F32, isOutput=False)
    kb_e = nc.declare_dram_parameter("kb", [128, NB], F32, isOutput=False)
    vb_e = nc.declare_dram_parameter("vb", [128, NB], F32, isOutput=False)
    qgfix_e = nc.declare_dram_parameter("qgfix", [128, NB], F32, isOutput=False)
    out_e = nc.declare_dram_parameter("out", [D, S], F32, isOutput=True)

    Exp = mybir.ActivationFunctionType.Exp
    Identity = mybir.ActivationFunctionType.Identity
    mult = mybir.AluOpType.mult
    add = mybir.AluOpType.add
    amax_op = mybir.AluOpType.max
    AxX = mybir.AxisListType.X

    with tile.TileContext(nc) as tc:
        from contextlib import ExitStack

        with ExitStack() as ctx:
            dramp = ctx.enter_context(tc.tile_pool(name="dram", bufs=1, space="DRAM"))
            wpool = ctx.enter_context(tc.tile_pool(name="w", bufs=16))
            wsm = ctx.enter_context(tc.tile_pool(name="wsm", bufs=24))
            xpool = ctx.enter_context(tc.tile_pool(name="x", bufs=16))
            stag = ctx.enter_context(tc.tile_pool(name="stag", bufs=8))
            ktp = ctx.enter_context(tc.tile_pool(name="kt", bufs=8))
            expp = ctx.enter_context(tc.tile_pool(name="exp", bufs=1))
            ttrs = ctx.enter_context(tc.tile_pool(name="ttrs", bufs=2))
            alp = ctx.enter_context(tc.tile_pool(name="al", bufs=1))
            qtb_p = ctx.enter_context(tc.tile_pool(name="qtb", bufs=2))
            vload_p = ctx.enter_context(tc.tile_pool(name="vload", bufs=16))
            qres_p = ctx.enter_context(tc.tile_pool(name="qres", bufs=8))
            ost_p = ctx.enter_context(tc.tile_pool(name="ost", bufs=8))
            consts = ctx.enter_context(tc.tile_pool(name="c", bufs=10))
            stats = ctx.enter_context(tc.tile_pool(name="st", bufs=10))
            pj_ps = ctx.enter_context(tc.tile_pool(name="pjps", bufs=4, space="PSUM"))
            al_ps = ctx.enter_context(tc.tile_pool(name="alps", bufs=2, space="PSUM"))

            qspill = dramp.tile([D, S], BF16, name="qspill")
            vspill = dramp.tile([D, S], BF16, name="vspill")
            ascr = dramp.tile([H, S], BF16, name="ascr")

            # ---- weights / consts to SBUF ----
            def load_w(src, nm):
                ts = []
                for kb in range(NB):
                    t = wpool.tile([128, D], BF16, tag="w", name=f"{nm}{kb}")
                    nc.sync.dma_start(out=t, in_=src[kb * 128:(kb + 1) * 128, :])
                    ts.append(t)
                return ts

            def load_wsm(src, nm):
                ts = []
                for kb in range(NB):
                    t = wsm.tile([128, H], BF16, tag="wsm", name=f"{nm}{kb}")
                    nc.sync.dma_start(out=t, in_=src[kb * 128:(kb + 1) * 128, :])
                    ts.append(t)
                return ts

            qw_sb = load_w(qw_e, "qw")
            wqx_sb = load_wsm(wqx_e, "wqx")
            kw_sb = load_w(kw_e, "kw")
            wks_sb = load_wsm(wks_e, "wks")

            def load_c(src, nm):
                t = consts.tile([128, NB], F32, name=nm)
                nc.gpsimd.dma_start(out=t, in_=src[:, :])
                return t

            qob_sb = load_c(qob_e, "qob_sb")
            kb_sb = load_c(kb_e, "kb_sb")
            vb_sb = load_c(vb_e, "vb_sb")
            qgfix_sb = load_c(qgfix_e, "qgfix_sb")

            alpha_sb = alp.tile([H, S], BF16, tag="al", name="alpha_sb")

            # ---- phase Q: q_hat = x_q @ q_w.T (+q_b+out_b), spill to DRAM;
            #      alpha = x_q @ (q_w.T @ wq_w.T)/8, fused from x tiles ----
            def x_chunk(src, n):
                ts = []
                for kb in range(NB):
                    t = xpool.tile([128, CH], BF16, tag="x", name=f"xt{kb}")
                    nc.sync.dma_start(
                        out=t,
                        in_=src[kb * 128:(kb + 1) * 128, n * CH:(n + 1) * CH],
                    )
                    ts.append(t)
                return ts

            for n in range(NCH):
                xt = x_chunk(xq_e, n)
                for m in range(NB):
                    ps = pj_ps.tile([128, CH], F32, tag="pjps", name="ps")
                    for kb in range(NB):
                        nc.tensor.matmul(
                            ps,
                            qw_sb[kb][:, m * 128:(m + 1) * 128],
                            xt[kb],
                            start=(kb == 0),
                            stop=(kb == NB - 1),
                        )
                    st = stag.tile([128, CH], BF16, tag="stag", name="st")
                    nc.scalar.activation(
                        st, ps, Identity, bias=qob_sb[:, m:m + 1], scale=1.0
                    )
                    nc.sync.dma_start(
                        out=qspill[m * 128:(m + 1) * 128, n * CH:(n + 1) * CH],
                        in_=st,
                    )
                aps = al_ps.tile([H, CH], F32, tag="alps", name="aps")
                for kb in range(NB):
                    nc.tensor.matmul(
                        aps, wqx_sb[kb], xt[kb],
                        start=(kb == 0), stop=(kb == NB - 1),
                    )
                nc.vector.tensor_copy(alpha_sb[:, n * CH:(n + 1) * CH], aps)

            # ---- softmax over tokens (rows = heads), normalized in place ----
            def softmax_rows(a_sb):
                amx = stats.tile([H, 1], F32, tag="st", name="amx")
                nc.vector.tensor_reduce(amx, a_sb, AxX, amax_op)
                namx = stats.tile([H, 1], F32, tag="st", name="namx")
                nc.vector.tensor_scalar_mul(namx, amx, -1.0)
                sume = stats.tile([H, 1], F32, tag="st", name="sume")
                nc.scalar.activation(
                    a_sb, a_sb, Exp, bias=namx[:, 0:1], scale=1.0, accum_out=sume
                )
                rsum = stats.tile([H, 1], F32, tag="st", name="rsum")
                nc.vector.reciprocal(rsum, sume)
                nc.vector.tensor_scalar_mul(a_sb, a_sb, rsum[:, 0:1])

            softmax_rows(alpha_sb)

            # replicate the 16 head rows to all 128 partitions via a
            # DRAM bounce + stride-0 broadcast DMA
            exp_bc = expp.tile([128, S], BF16, tag="exp", name="exp_bc")

            def bcast_rows(a_sb, dst):
                nc.gpsimd.dma_start(out=ascr[:, :], in_=a_sb)
                a = ascr[:, :]
                src = bass.AP(tensor=a.tensor, offset=a.offset,
                              ap=[[0, 8]] + list(a.ap))
                nc.gpsimd.dma_start(out=dst, in_=src)

            bcast_rows(alpha_sb, exp_bc)

            # ---- pool1: q_global = sum_t alphas * q_hat  (stream q_hat) ----
            qg = consts.tile([128, NB], F32, name="qg")
            PCH = 1024
            NPC = S // PCH
            for m in range(NB):
                qtb = qtb_p.tile([128, S], BF16, tag="qtb", name="qtb")
                nc.sync.dma_start(out=qtb, in_=qspill[m * 128:(m + 1) * 128, :])
                parts = stats.tile([128, NPC], F32, tag="part", name="parts")
                for j in range(NPC):
                    sc = ttrs.tile([128, PCH], BF16, tag="ttrs", name="sc")
                    nc.vector.tensor_tensor(
                        sc, qtb[:, j * PCH:(j + 1) * PCH],
                        exp_bc[:, j * PCH:(j + 1) * PCH], mult
                    )
                    nc.vector.tensor_reduce(parts[:, j:j + 1], sc, AxX, add)
                nc.vector.tensor_reduce(qg[:, m:m + 1], parts, AxX, add)
            # q_global fixup (pooled q_hat includes q_b+out_b; true qg needs
            # +q_b only): qg += -out_b
            nc.vector.tensor_tensor(qg, qg, qgfix_sb, add)

            # wk_qg[d, h] = wk_s[d, h] * qg[d]
            wkqg_sb = []
            for kb in range(NB):
                t = wsm.tile([128, H], BF16, tag="wsm", name=f"wkqg{kb}")
                nc.vector.tensor_scalar_mul(t, wks_sb[kb], qg[:, kb:kb + 1])
                wkqg_sb.append(t)

            # ---- phase K: k_tilde resident in SBUF ----
            kt_sb = [
                ktp.tile([128, S], BF16, tag="kt", name=f"kt{i}") for i in range(NB)
            ]
            for n in range(NCH):
                xt = x_chunk(xkv_e, n)
                for m in range(NB):
                    ps = pj_ps.tile([128, CH], F32, tag="pjps", name="ps")
                    for kb in range(NB):
                        nc.tensor.matmul(
                            ps,
                            kw_sb[kb][:, m * 128:(m + 1) * 128],
                            xt[kb],
                            start=(kb == 0),
                            stop=(kb == NB - 1),
                        )
                    nc.scalar.copy(kt_sb[m][:, n * CH:(n + 1) * CH], ps)

            # prefetch v/out weights into freed w slots
            vw_sb = load_w(vw_e, "vw")
            ow_sb = load_w(ow_e, "ow")

            # ---- beta = wk_qg @ k_tilde ----
            beta_sb = alp.tile([H, S], BF16, tag="al", name="beta_sb")
            for n in range(NCH):
                bps = al_ps.tile([H, CH], F32, tag="alps", name="bps")
                for mb in range(NB):
                    nc.tensor.matmul(
                        bps, wkqg_sb[mb], kt_sb[mb][:, n * CH:(n + 1) * CH],
                        start=(mb == 0), stop=(mb == NB - 1),
                    )
                nc.vector.tensor_copy(beta_sb[:, n * CH:(n + 1) * CH], bps)

            softmax_rows(beta_sb)
            exp_bc2 = expp.tile([128, S], BF16, tag="exp", name="exp_bc2")
            bcast_rows(beta_sb, exp_bc2)

            # ---- pool2 + k_global = qg * (pool + k_b); vbkg = v_b * kg ----
            kg = consts.tile([128, NB], F32, name="kg")
            for m in range(NB):
                parts2 = stats.tile([128, NPC], F32, tag="part", name="parts2")
                for j in range(NPC):
                    sc2 = ttrs.tile([128, PCH], BF16, tag="ttrs", name="sc2")
                    nc.vector.tensor_tensor(
                        sc2,
                        kt_sb[m][:, j * PCH:(j + 1) * PCH],
                        exp_bc2[:, j * PCH:(j + 1) * PCH],
                        mult,
                    )
                    nc.vector.tensor_reduce(parts2[:, j:j + 1], sc2, AxX, add)
                nc.vector.tensor_reduce(kg[:, m:m + 1], parts2, AxX, add)
            nc.vector.tensor_tensor(kg, kg, kb_sb, add)
            nc.vector.tensor_tensor(kg, kg, qg, mult)
            vbkg = consts.tile([128, NB], F32, name="vbkg")
            nc.vector.tensor_tensor(vbkg, vb_sb, kg, mult)

            # ---- phase V: v_tilde spilled to DRAM (raw; scaled on reload) ----
            for n in range(NCH):
                xt = x_chunk(xkv_e, n)
                for m in range(NB):
                    ps = pj_ps.tile([128, CH], F32, tag="pjps", name="ps")
                    for kb in range(NB):
                        nc.tensor.matmul(
                            ps,
                            vw_sb[kb][:, m * 128:(m + 1) * 128],
                            xt[kb],
                            start=(kb == 0),
                            stop=(kb == NB - 1),
                        )
                    st = stag.tile([128, CH], BF16, tag="stag", name="st")
                    nc.scalar.copy(st, ps)
                    nc.sync.dma_start(
                        out=vspill[m * 128:(m + 1) * 128, n * CH:(n + 1) * CH],
                        in_=st,
                    )

            # ---- out = q_hat + (v_tilde*kg + v_b*kg) @ out_w.T ----
            for n in range(NCH):
                vts = []
                for kb in range(NB):
                    vl = vload_p.tile([128, CH], BF16, tag="vload", name=f"vl{kb}")
                    nc.sync.dma_start(
                        out=vl,
                        in_=vspill[kb * 128:(kb + 1) * 128, n * CH:(n + 1) * CH],
                    )
                    nc.scalar.activation(
                        vl, vl, Identity,
                        bias=vbkg[:, kb:kb + 1], scale=kg[:, kb:kb + 1],
                    )
                    vts.append(vl)
                for m in range(NB):
                    ps = pj_ps.tile([128, CH], F32, tag="pjps", name="ps")
                    for kb in range(NB):
                        nc.tensor.matmul(
                            ps,
                            ow_sb[kb][:, m * 128:(m + 1) * 128],
                            vts[kb],
                            start=(kb == 0),
                            stop=(kb == NB - 1),
                        )
                    qres = qres_p.tile([128, CH], BF16, tag="qres", name="qres")
                    nc.sync.dma_start(
                        out=qres,
                        in_=qspill[m * 128:(m + 1) * 128, n * CH:(n + 1) * CH],
                    )
                    ost = ost_p.tile([128, CH], F32, tag="ost", name="ost")
                    nc.vector.tensor_tensor(ost, ps, qres, add)
                    nc.sync.dma_start(
                        out=out_e[m * 128:(m + 1) * 128, n * CH:(n + 1) * CH],
                        in_=ost,
                    )

    legalize_waits(nc)
    return nc


_NC_CACHE = None


def kernel(x_q, x_kv, q_w, k_w, v_w, wq_w, wk_w, out_w,
           q_b, k_b, v_b, wq_b, wk_b, out_b):
    global _NC_CACHE
    if _NC_CACHE is None:
        _NC_CACHE = build_kernel()
    nc = _NC_CACHE

    x_q = np.asarray(x_q, np.float32)
    x_kv = np.asarray(x_kv, np.float32)
    q_w = np.asarray(q_w, np.float32)
    k_w = np.asarray(k_w, np.float32)
    v_w = np.asarray(v_w, np.float32)
    wq_w = np.asarray(wq_w, np.float32)
    wk_w = np.asarray(wk_w, np.float32)
    out_w = np.asarray(out_w, np.float32)
    q_b = np.asarray(q_b, np.float32)
    k_b = np.asarray(k_b, np.float32)
    v_b = np.asarray(v_b, np.float32)
    out_b = np.asarray(out_b, np.float32)
    # wq_b / wk_b shift alpha/beta by a per-head constant -> cancel in the
    # token softmax; mathematically irrelevant.

    in_maps = make_in_maps(x_q, x_kv, q_w, k_w, v_w, wq_w, wk_w, out_w,
                           q_b, k_b, v_b, out_b)
    res = run_bass_kernel_spmd(nc, in_maps, list(range(N_CORES)))
    out = np.empty((B, S, D), np.float32)
    for c in range(N_CORES):
        out[c][:, P_IDX] = res.results[c]["out"].T
    return out


def make_in_maps(x_q, x_kv, q_w, k_w, v_w, wq_w, wk_w, out_w,
                 q_b, k_b, v_b, out_b):
    P = P_IDX
    shared = dict(
        qw=np.ascontiguousarray(q_w.T[:, P]).astype(NPBF16),
        kw=np.ascontiguousarray(k_w.T[:, P]).astype(NPBF16),
        vw=np.ascontiguousarray(v_w.T[:, P]).astype(NPBF16),
        ow=np.ascontiguousarray(out_w.T[np.ix_(P, P)]).astype(NPBF16),
        wqx=np.ascontiguousarray((q_w.T @ wq_w.T) / 8.0).astype(NPBF16),
        wks=np.ascontiguousarray(wk_w[:, P].T / 8.0).astype(NPBF16),
        qob=np.ascontiguousarray((q_b + out_b)[P].reshape(NB, 128).T).astype(np.float32),
        kb=np.ascontiguousarray(k_b[P].reshape(NB, 128).T).astype(np.float32),
        vb=np.ascontiguousarray(v_b[P].reshape(NB, 128).T).astype(np.float32),
        qgfix=np.ascontiguousarray((-out_b)[P].reshape(NB, 128).T).astype(np.float32),
    )
    in_maps = []
    for c in range(N_CORES):
        m = dict(shared)
        m["xq"] = x_q[c].T.astype(NPBF16)
        m["xkv"] = x_kv[c].T.astype(NPBF16)
        in_maps.append(m)
    return in_maps
